# revision 44
# baseline (speedup 1.0000x reference)
"""DalleSelfAttention Trainium2 kernel (8 NeuronCores).

Sharding: tensor-parallel over heads (4 groups of 4 heads) x data-parallel
over batch (2), i.e. core c = b*4 + hg computes, for batch b, the partial
attention output of heads [4*hg, 4*hg+4), including its slice of the QKV
projection and its partial of the output projection. The host sums the 4
partials per batch and adds the output bias.

Device-side math per core (S=2048 seq, d=128 head dim, 4 heads):
  qT/kT = (x Wq^T)^T etc. in [d, s] layout, V in [s, d] layout.
  scores^T[k, q] = kT-slices.T @ qT  (PE, bf16)
  E = exp(scores^T / sqrt(d)) * mask^T  (ACT exp; DVE mul only on partial
      mask blocks; zero blocks are skipped outright)
  ctx^T[d, q] = sum_k V-slices.T @ E   (PE, bf16)
  r[q]: E chunk-pair tree reduction on DVE down to one [128,512] tile,
      then a single ones.T @ root matmul (PE) replicates r across
      partitions.  This keeps the softmax-denominator work off the PE
      streaming path (the PE is the bottleneck engine).
  ctxn^T = ctx^T * (1/r)               (DVE, bf16)
  out_partial[q, n] = sum_h ctxn_h^T.T @ Wout_h^T  (PE, bf16; written to
      DRAM as bf16 to halve the output-drain DMA)
The pb-relax max-rescaling of the reference cancels exactly under softmax
shift invariance; with these inputs scores are O(1) so exp never overflows,
and masked entries are exactly zeroed by the multiplicative mask.

All device inputs are pre-packed on the host into the exact per-partition
SBUF layouts, so every DMA is a contiguous [128, N] copy. Only mask chunks
that are partially masked are shipped (fully-ones chunks need no multiply,
all-zero chunks are skipped). Attention is software-pipelined over
(query-block, head) with big and small query blocks interleaved so the ACT
exp stream for full-length blocks overlaps the PE-heavy small-block
iterations.
"""

import numpy as np
import ml_dtypes

H = 2048
NH = 16
HN = 128
B = 2
S = 2048
NG = 4            # head groups (tensor-parallel degree)
DG = 512          # q/k/v dims per group
P = 128
QBS = 512
SCALE = 1.0 / float(np.sqrt(128.0))

_COMPILED = {}


def _build(keep):
    from contextlib import ExitStack
    import concourse.tile as tile
    from concourse import bacc, mybir

    f32 = mybir.dt.float32
    bf16 = mybir.dt.bfloat16
    Identity = mybir.ActivationFunctionType.Identity
    Exp = mybir.ActivationFunctionType.Exp

    # per-qb offsets (in 128-col sub-blocks) into the packed mask stream
    mask_cols = [sum(len(parts) for _kc, _vs, parts in kcs) for kcs in keep]
    mask_off = [0]
    for qb in range(4):
        mask_off.append(mask_off[-1] + mask_cols[qb])
    n_mask_chunks = mask_off[-1]

    nc = bacc.Bacc("TRN2", target_bir_lowering=False, debug=False)
    xp = nc.dram_tensor("xp", [P, 4 * 16 * 512], bf16, kind="ExternalInput").ap()
    wq = nc.dram_tensor("wq", [P, 4 * 16 * P], bf16, kind="ExternalInput").ap()
    wk = nc.dram_tensor("wk", [P, 4 * 16 * P], bf16, kind="ExternalInput").ap()
    wv = nc.dram_tensor("wv", [P, 16 * DG], bf16, kind="ExternalInput").ap()
    wo = nc.dram_tensor("wo", [P, NG * H], bf16, kind="ExternalInput").ap()
    maskp = nc.dram_tensor("maskp", [P, max(n_mask_chunks, 1) * P], bf16,
                           kind="ExternalInput").ap()
    bqk = nc.dram_tensor("bqk", [P, 8], f32, kind="ExternalInput").ap()
    bvb = nc.dram_tensor("bvb", [P, DG], f32, kind="ExternalInput").ap()
    outp = nc.dram_tensor("outp", [S, H], bf16, kind="ExternalOutput").ap()

    NHC = H // P      # 16 contraction chunks over hidden
    NSQ = 4           # seq quarters for the projection phase
    SQ = S // NSQ     # 512
    NKC = S // P      # 16 key chunks
    NQB = 4           # query blocks
    QB = QBS          # 512
    ND = DG // P      # 4 d-chunks per section == heads per group

    # big/small interleave: full-length blocks alternate with short ones
    qb_iters = []
    for pair in ((3, 0), (2, 1)):
        for h in range(NG):
            qb_iters.append((pair[0], h))
            qb_iters.append((pair[1], h))

    with tile.TileContext(nc) as tc, ExitStack() as ctx:
        persist = ctx.enter_context(tc.tile_pool(name="persist", bufs=1))
        qT = persist.tile([P, NG * S], bf16)      # [d, h*S + s]
        kT = persist.tile([P, NG * S], bf16)      # [d, h*S + s]
        V = persist.tile([P, NKC * DG], bf16)     # [s, st*DG + d]
        woTs = persist.tile([P, NG * H], bf16)    # [d, h*H + n]
        bqk_s = persist.tile([P, 8], f32)
        bvb_s = persist.tile([P, DG], f32)
        ones = persist.tile([P, P], bf16)

        nc.vector.memset(ones[:], 1.0)

        mpool = ctx.enter_context(tc.tile_pool(name="mask", bufs=2))
        mask_tiles = {}

        def load_mask(qb):
            ncols = max(mask_cols[qb], 1)
            mtile = mpool.tile([P, ncols * P], bf16, tag="mt", name=f"mt{qb}")
            if mask_cols[qb]:
                nc.sync.dma_start(
                    out=mtile[:, :mask_cols[qb] * P],
                    in_=maskp[:, mask_off[qb] * P:mask_off[qb + 1] * P])
            mask_tiles[qb] = mtile

        # ---- Phase A: QKV projection ----
        # Weight slices stay resident in SBUF; x^T streams in seq quarters.
        with tc.tile_pool(name="wA", bufs=1) as wapool, \
             tc.tile_pool(name="xq", bufs=4) as xpool, \
             tc.tile_pool(name="pv_acc", bufs=1, space="PSUM") as pvp, \
             tc.tile_pool(name="pqk_acc", bufs=4, space="PSUM") as pqk:
            xq_tiles = {}

            def load_xq(sq, hf, split=1):
                t = xpool.tile([P, (NHC // 2) * SQ], bf16, tag="xq",
                               name=f"xq{sq}_{hf}")
                base = (sq * 2 + hf) * 4096
                step = 4096 // split
                for i in range(split):
                    nc.sync.dma_start(
                        out=t[:, i * step:(i + 1) * step],
                        in_=xp[:, base + i * step:base + (i + 1) * step])
                xq_tiles[(sq, hf)] = t

            # first weight chunk + first x quarter split fine so the first
            # matmul's wait covers ~100KB, not megabytes; non-critical loads
            # (biases, masks) are issued later so their ring entries don't
            # steal startup bandwidth
            # layout [h, hf*4096 + dc*1024 + (hc%8)*128 + d]: each hidden
            # half's weights contiguous, so DMAs land in need-order
            wq_sb = wapool.tile([P, ND * NHC * P], bf16)
            wv_sb = wapool.tile([P, NHC * DG], bf16)   # [h, hc*DG + d]
            wk_sb = wapool.tile([P, ND * NHC * P], bf16)
            nc.sync.dma_start(out=wq_sb[:, :512], in_=wq[:, :512])
            nc.sync.dma_start(out=wq_sb[:, 512:1024], in_=wq[:, 512:1024])
            load_xq(0, 0, split=4)
            nc.sync.dma_start(out=wq_sb[:, 1024:2048], in_=wq[:, 1024:2048])
            nc.sync.dma_start(out=wq_sb[:, 2048:4096], in_=wq[:, 2048:4096])
            nc.sync.dma_start(out=wv_sb[:, :4096], in_=wv[:, :4096])
            load_xq(0, 1, split=2)
            nc.sync.dma_start(out=wq_sb[:, 4096:], in_=wq[:, 4096:])
            nc.sync.dma_start(out=bqk_s[:], in_=bqk)
            nc.sync.dma_start(out=wv_sb[:, 4096:], in_=wv[:, 4096:])
            nc.sync.dma_start(out=bvb_s[:], in_=bvb)
            nc.sync.dma_start(out=wk_sb[:], in_=wk)

            for sq in range(NSQ):
                if sq == 2:
                    load_mask(3)
                    load_mask(0)
                for hf in range(2):
                    if (sq, hf) not in xq_tiles:
                        load_xq(sq, hf)
                xh = [xq_tiles.pop((sq, 0)), xq_tiles.pop((sq, 1))]
                if sq + 1 < NSQ:
                    load_xq(sq + 1, 0)
                    load_xq(sq + 1, 1)

                def xslice(hc, lo, hi):
                    return xh[hc // 8][:, (hc % 8) * SQ + lo:(hc % 8) * SQ + hi]

                # q and v accumulate in two hidden-dim halves so the first
                # matmuls only depend on the first x half-tile (faster ramp)
                qaccs = [pqk.tile([P, SQ], f32, tag="qkacc",
                                  name=f"qkacc{sq}_0_{dc}")
                         for dc in range(ND)]
                vaccs = [pvp.tile([P, DG], f32, tag=f"vacc{st}",
                                  name=f"vacc{st}_{sq}")
                         for st in range(4)]

                def q_half(hcs):
                    for dc in range(ND):
                        for hc in hcs:
                            col = (hc // 8) * 4096 + dc * 1024 + (hc % 8) * P
                            nc.tensor.matmul(
                                qaccs[dc][:],
                                lhsT=wq_sb[:, col: col + P],
                                rhs=xslice(hc, 0, SQ),
                                start=(hc == 0), stop=(hc == NHC - 1),
                            )
                        if hcs[-1] == NHC - 1:
                            nc.scalar.activation(
                                out=qT[:, dc * S + sq * SQ: dc * S + (sq + 1) * SQ],
                                in_=qaccs[dc][:], func=Identity,
                                bias=bqk_s[:, dc: dc + 1], scale=1.0,
                            )

                def v_half(hcs):
                    for hc in hcs:
                        for st in range(4):
                            nc.tensor.matmul(
                                vaccs[st][:],
                                lhsT=xslice(hc, st * P, (st + 1) * P),
                                rhs=wv_sb[:, hc * DG:(hc + 1) * DG],
                                start=(hc == 0), stop=(hc == NHC - 1),
                            )
                    if hcs[-1] == NHC - 1:
                        for st in range(4):
                            stg = sq * 4 + st
                            nc.vector.tensor_add(
                                V[:, stg * DG:(stg + 1) * DG], vaccs[st][:],
                                bvb_s[:])

                def k_sec():
                    for dc in range(ND):
                        acc = pqk.tile([P, SQ], f32, tag="qkacc",
                                       name=f"qkacc{sq}_1_{dc}")
                        for hc in range(NHC):
                            col = (hc // 8) * 4096 + dc * 1024 + (hc % 8) * P
                            nc.tensor.matmul(
                                acc[:],
                                lhsT=wk_sb[:, col: col + P],
                                rhs=xslice(hc, 0, SQ),
                                start=(hc == 0), stop=(hc == NHC - 1),
                            )
                        nc.scalar.activation(
                            out=kT[:, dc * S + sq * SQ: dc * S + (sq + 1) * SQ],
                            in_=acc[:], func=Identity,
                            bias=bqk_s[:, 4 + dc: 4 + dc + 1], scale=1.0,
                        )

                lo, hi = list(range(NHC // 2)), list(range(NHC // 2, NHC))
                if sq == NSQ - 1:
                    # k first on the last quarter so phase B's first scores
                    # matmuls (which need all of kT) unblock sooner
                    k_sec()
                    q_half(lo)
                    q_half(hi)
                    v_half(lo)
                    v_half(hi)
                else:
                    q_half(lo)
                    v_half(lo)
                    q_half(hi)
                    v_half(hi)
                    k_sec()

        # ---- Phase B+C: attention + output projection ----
        # Software-pipelined over (query-block, head): the QK->exp->mask
        # chain for iteration i+1 is emitted before the PV/r consumption of
        # iteration i.
        with tc.tile_pool(name="epool", bufs=4) as epool, \
             tc.tile_pool(name="cpool", bufs=2) as cpool, \
             tc.tile_pool(name="spool", bufs=2) as spool, \
             tc.tile_pool(name="tpool", bufs=2) as tpool, \
             tc.tile_pool(name="opool", bufs=3) as opool, \
             tc.tile_pool(name="ps_s", bufs=2, space="PSUM") as ps_s, \
             tc.tile_pool(name="ps_cr", bufs=1, space="PSUM") as ps_cr, \
             tc.tile_pool(name="ps_o", bufs=2, space="PSUM") as ps_o:
            e_tiles = {}
            e_roots = {}
            ctx_tiles = {}

            def rowsum_tree(qb, h, E, nchunks):
                """Pairwise-reduce E's chunks on DVE down to one [P,QB]
                bf16 tile; returns the root AP."""
                if nchunks == 1:
                    return E[:, :QB]
                scrA = tpool.tile([P, 8 * QB], bf16, tag="trA",
                                  name=f"trA{qb}_{h}")
                scrB = tpool.tile([P, 4 * QB], bf16, tag="trB",
                                  name=f"trB{qb}_{h}")
                src, srcn = E, nchunks
                dsts = [scrA, scrB]
                lvl = 0
                while srcn > 1:
                    half = srcn // 2
                    odd = srcn % 2
                    dst = dsts[lvl % 2]
                    nc.vector.tensor_add(
                        dst[:, :half * QB],
                        src[:, :half * QB],
                        src[:, half * QB:2 * half * QB])
                    if odd:
                        nc.vector.tensor_add(
                            dst[:, :QB], dst[:, :QB],
                            src[:, 2 * half * QB:(2 * half + 1) * QB])
                    src, srcn = dst, half
                    lvl += 1
                return src[:, :QB]

            def produce(qb, h):
                # mask prefetch: a slot is reused only after every read of
                # its previous tile has already been emitted
                if (qb, h) == (0, 3):
                    load_mask(2)
                if (qb, h) == (2, 0):
                    load_mask(1)
                mt = mask_tiles[qb]
                kcs = keep[qb]
                E = epool.tile([P, len(kcs) * QB], bf16, tag="E",
                               name=f"E{qb}_{h}")
                # zero the sub-128-granular invalid head of partial chunks
                # so the rowsum tree and the first PV stream read zeros
                for pos, (kc, vs, parts) in enumerate(kcs):
                    if vs:
                        nc.vector.memset(E[:, pos * QB:pos * QB + vs * P], 0.0)
                mpos = 0
                pos = 0
                n = len(kcs)
                while pos < n:
                    # pair-batch consecutive full chunks; partial chunks go
                    # one at a time at their valid width
                    if kcs[pos][1] == 0 and pos + 1 < n and kcs[pos + 1][1] == 0:
                        grp = [pos, pos + 1]
                    else:
                        grp = [pos]
                    ps = ps_s.tile([P, len(grp) * QB], f32, tag="ps",
                                   name=f"ps{qb}_{h}_{pos}")
                    for gi, p_ in enumerate(grp):
                        kc, vs, _parts = kcs[p_]
                        nc.tensor.matmul(
                            ps[:, gi * QB + vs * P:(gi + 1) * QB],
                            lhsT=kT[:, h * S + kc * P: h * S + (kc + 1) * P],
                            rhs=qT[:, h * S + qb * QB + vs * P:
                                   h * S + (qb + 1) * QB],
                            start=True, stop=True,
                        )
                    vs0 = kcs[grp[0]][1]
                    if len(grp) == 2:
                        nc.scalar.activation(
                            out=E[:, pos * QB:(pos + 2) * QB], in_=ps[:],
                            func=Exp, scale=SCALE)
                    else:
                        nc.scalar.activation(
                            out=E[:, pos * QB + vs0 * P:(pos + 1) * QB],
                            in_=ps[:, vs0 * P:QB], func=Exp, scale=SCALE)
                    for p_ in grp:
                        kc, vs, parts = kcs[p_]
                        for j in parts:
                            nc.vector.tensor_mul(
                                E[:, p_ * QB + j * P:p_ * QB + (j + 1) * P],
                                E[:, p_ * QB + j * P:p_ * QB + (j + 1) * P],
                                mt[:, mpos * P:(mpos + 1) * P])
                            mpos += 1
                    pos += len(grp)
                e_tiles[(qb, h)] = E
                e_roots[(qb, h)] = rowsum_tree(qb, h, E, n)

            def consume(qb, h):
                kcs = keep[qb]
                E = e_tiles.pop((qb, h))
                if h == 0:
                    ctx_tiles[qb] = cpool.tile(
                        [P, NG * QB], bf16, tag="ctxn", name=f"ctxn{qb}")
                ctxn = ctx_tiles[qb]
                pc = ps_cr.tile([P, QB], f32, tag="ctx", name=f"pc{qb}_{h}")
                pr = ps_cr.tile([P, QB], f32, tag="r", name=f"pr{qb}_{h}")
                last = len(kcs) - 1
                for pos, (kc, vs, _parts) in enumerate(kcs):
                    # first chunk streams full width (its invalid head is
                    # zeroed) so the whole PSUM tile gets initialized
                    w = 0 if pos == 0 else vs * P
                    nc.tensor.matmul(
                        pc[:, w:],
                        lhsT=V[:, kc * DG + h * P: kc * DG + (h + 1) * P],
                        rhs=E[:, pos * QB + w:(pos + 1) * QB],
                        start=(pos == 0), stop=(pos == last),
                    )
                root = e_roots.pop((qb, h))
                nc.tensor.matmul(pr[:], lhsT=ones[:], rhs=root,
                                 start=True, stop=True)
                rinv = spool.tile([P, QB], f32, tag="rinv", name=f"rinv{qb}_{h}")
                nc.vector.reciprocal_approx_fast(out=rinv[:], in_=pr[:])
                nc.vector.tensor_mul(
                    ctxn[:, h * QB:(h + 1) * QB], pc[:], rinv[:])

            def out_proj(qb):
                ctxn = ctx_tiles.pop(qb)
                for st in range(4):
                    ot = opool.tile([P, H], bf16, tag="ot", name=f"ot{qb}_{st}")
                    row = (qb * 4 + st) * P
                    for n in range(4):
                        po = ps_o.tile([P, 512], f32, tag="po",
                                       name=f"po{qb}_{st}_{n}")
                        for h in range(NG):
                            nc.tensor.matmul(
                                po[:],
                                lhsT=ctxn[:, h * QB + st * P: h * QB + (st + 1) * P],
                                rhs=woTs[:, h * H + n * 512: h * H + (n + 1) * 512],
                                start=(h == 0), stop=(h == NG - 1),
                            )
                        if n % 2 == 0:
                            nc.vector.tensor_copy(
                                ot[:, n * 512:(n + 1) * 512], po[:])
                        else:
                            nc.scalar.copy(ot[:, n * 512:(n + 1) * 512], po[:])
                        # half-tile DMAs so the drain overlaps the copies
                        if n == 1:
                            nc.sync.dma_start(out=outp[row:row + P, :1024],
                                              in_=ot[:, :1024])
                        elif n == 3:
                            nc.sync.dma_start(out=outp[row:row + P, 1024:],
                                              in_=ot[:, 1024:])

            nc.sync.dma_start(out=woTs[:], in_=wo)
            AHEAD = 3
            for j in range(AHEAD):
                produce(*qb_iters[j])
            for i, (qb, h) in enumerate(qb_iters):
                if i + AHEAD < len(qb_iters):
                    produce(*qb_iters[i + AHEAD])
                consume(qb, h)
                if h == NG - 1:
                    out_proj(qb)
    nc.compile()
    return nc


def _keep_lists(mask):
    """Per query-block: list of (kc, valid_start, parts) for key chunks
    whose [128k x 512q] mask block is not identically zero, classified at
    [128k x 128q] sub-block granularity:
      valid_start: first 128-q sub-block with any nonzero mask (the chunk's
        scores/E left of it are never computed; they are memset to zero).
      parts: sub-blocks >= valid_start that are not all-ones (these get the
        elementwise mask multiply).
    Exact for any float mask."""
    mt = mask.T.reshape(S // P, P, 4, 4, P)  # [kc, k, qb, qsub, q]
    bmax = mt.max(axis=(1, 4))  # [kc, qb, qsub]
    bmin = mt.min(axis=(1, 4))
    keep = []
    for qb in range(4):
        kcs = []
        for kc in range(S // P):
            zero = [bmax[kc, qb, j] == 0.0 for j in range(4)]
            onesb = [bmin[kc, qb, j] == 1.0 and bmax[kc, qb, j] == 1.0
                     for j in range(4)]
            if all(zero):
                continue
            vs = next(j for j in range(4) if not zero[j])
            parts = tuple(j for j in range(vs, 4) if not onesb[j])
            kcs.append((kc, vs, parts))
        keep.append(kcs if kcs else [(qb * 4, 0, (0, 1, 2, 3))])
    return keep


def _get_compiled(mask):
    keep = _keep_lists(mask)
    key = tuple(tuple(k) for k in keep)
    if key not in _COMPILED:
        _COMPILED[key] = (_build(keep), keep)
    return _COMPILED[key]


def _pack_pt(arr, inner):
    """[nchunk*128, n*inner] -> [128, n*nchunk*inner] with layout
    [p, n_idx*nchunk*inner + chunk*inner + i]."""
    nchunk = arr.shape[0] // P
    n = arr.shape[1] // inner
    return np.ascontiguousarray(
        arr.reshape(nchunk, P, n, inner).transpose(1, 2, 0, 3).reshape(
            P, n * nchunk * inner))


def _pack_qk(arr):
    """[2048h, 512d] -> [128, 8192] with layout
    [p, hf*4096 + dc*1024 + hcl*128 + d] (hc = hf*8 + hcl)."""
    return np.ascontiguousarray(
        arr.reshape(2, 8, P, 4, P).transpose(2, 0, 3, 1, 4).reshape(P, 8192))


def _in_maps(hidden_states, ltor_mask, W_qkv, b_qkv, W_out):
    bf = ml_dtypes.bfloat16
    hs = np.asarray(hidden_states, np.float32)
    mask = np.asarray(ltor_mask, np.float32).reshape(S, S)
    W_qkv = np.asarray(W_qkv, np.float32)
    b_qkv = np.asarray(b_qkv, np.float32)
    W_out = np.asarray(W_out, np.float32)

    keep = _keep_lists(mask)
    # mask^T packed: only the partial [128x128] sub-blocks, in keep order
    mT = mask.T.astype(bf)  # [k, q]
    cols = []
    for qb in range(4):
        for kc, _vs, parts in keep[qb]:
            for j in parts:
                cols.append(mT[kc * P:(kc + 1) * P,
                               qb * QBS + j * P:qb * QBS + (j + 1) * P])
    if cols:
        maskp = np.ascontiguousarray(
            np.concatenate(cols, axis=1).reshape(P, -1))
    else:
        maskp = np.zeros((P, P), bf)

    Wq, Wk, Wv = W_qkv[:H], W_qkv[H:2 * H], W_qkv[2 * H:]
    bq, bk, bv = b_qkv[:H], b_qkv[H:2 * H], b_qkv[2 * H:]

    # x^T packed per seq quarter: [p, sq*8192 + hc*512 + s]
    xps = [_pack_pt(hs[b].T.astype(bf), 512) for b in range(B)]
    in_maps = []
    for c in range(8):
        b, hg = divmod(c, NG)
        sl = slice(hg * DG, (hg + 1) * DG)
        bqk_np = np.concatenate(
            [bq[sl].reshape(4, P).T, bk[sl].reshape(4, P).T], axis=1)
        in_maps.append({
            "xp": xps[b],
            "wq": _pack_qk(Wq[sl].T.astype(bf)),  # [p, hf*4096+dc*1024+hcl*128+d]
            "wk": _pack_qk(Wk[sl].T.astype(bf)),
            "wv": _pack_pt(Wv[sl].T.astype(bf), DG),  # [p, hc*512+d]
            "wo": _pack_pt(W_out[:, sl].T.astype(bf), H),  # [p, h*2048+n]
            "maskp": maskp,
            "bqk": np.ascontiguousarray(bqk_np, dtype=np.float32),
            "bvb": np.ascontiguousarray(
                np.broadcast_to(bv[sl][None, :], (P, DG)), dtype=np.float32),
        })
    return in_maps


def kernel(hidden_states, ltor_mask, W_qkv, b_qkv, W_out, b_out):
    import os
    os.environ["BASS_NEVER_TRACE"] = "1"  # NTFF hook absent in this image
    from concourse.bass_utils import run_bass_kernel_spmd

    mask = np.asarray(ltor_mask, np.float32).reshape(S, S)
    nc, _ = _get_compiled(mask)
    in_maps = _in_maps(hidden_states, ltor_mask, W_qkv, b_qkv, W_out)
    res = run_bass_kernel_spmd(nc, in_maps, core_ids=list(range(8)))
    b_out = np.asarray(b_out, np.float32)
    out = np.empty((B, S, H), np.float32)
    for b in range(B):
        acc = res.results[NG * b]["outp"].astype(np.float32)
        for hg in range(1, NG):
            acc += res.results[NG * b + hg]["outp"].astype(np.float32)
        out[b] = acc + b_out[None, :]
    return out


# revision 45
# speedup vs baseline: 1.0046x; 1.0046x over previous
"""DalleSelfAttention Trainium2 kernel (8 NeuronCores).

Sharding: tensor-parallel over heads (4 groups of 4 heads) x data-parallel
over batch (2), i.e. core c = b*4 + hg computes, for batch b, the partial
attention output of heads [4*hg, 4*hg+4), including its slice of the QKV
projection and its partial of the output projection. The host sums the 4
partials per batch and adds the output bias.

Device-side math per core (S=2048 seq, d=128 head dim, 4 heads):
  qT/kT = (x Wq^T)^T etc. in [d, s] layout, V in [s, d] layout.
  scores^T[k, q] = kT-slices.T @ qT  (PE, bf16)
  E = exp(scores^T / sqrt(d)) * mask^T  (ACT exp; DVE mul only on partial
      mask blocks; zero blocks are skipped outright)
  ctx^T[d, q] = sum_k V-slices.T @ E   (PE, bf16)
  r[q]: E chunk-pair tree reduction on DVE down to one [128,512] tile,
      then a single ones.T @ root matmul (PE) replicates r across
      partitions.  This keeps the softmax-denominator work off the PE
      streaming path (the PE is the bottleneck engine).
  ctxn^T = ctx^T * (1/r)               (DVE, bf16)
  out_partial[q, n] = sum_h ctxn_h^T.T @ Wout_h^T  (PE, bf16; written to
      DRAM as bf16 to halve the output-drain DMA)
The pb-relax max-rescaling of the reference cancels exactly under softmax
shift invariance; with these inputs scores are O(1) so exp never overflows,
and masked entries are exactly zeroed by the multiplicative mask.

All device inputs are pre-packed on the host into the exact per-partition
SBUF layouts, so every DMA is a contiguous [128, N] copy. Only mask chunks
that are partially masked are shipped (fully-ones chunks need no multiply,
all-zero chunks are skipped). Attention is software-pipelined over
(query-block, head) with big and small query blocks interleaved so the ACT
exp stream for full-length blocks overlaps the PE-heavy small-block
iterations.
"""

import numpy as np
import ml_dtypes

H = 2048
NH = 16
HN = 128
B = 2
S = 2048
NG = 4            # head groups (tensor-parallel degree)
DG = 512          # q/k/v dims per group
P = 128
QBS = 512
SCALE = 1.0 / float(np.sqrt(128.0))

_COMPILED = {}


def _build(keep):
    from contextlib import ExitStack
    import concourse.tile as tile
    from concourse import bacc, mybir

    f32 = mybir.dt.float32
    bf16 = mybir.dt.bfloat16
    Identity = mybir.ActivationFunctionType.Identity
    Exp = mybir.ActivationFunctionType.Exp

    # per-qb offsets (in 128-col sub-blocks) into the packed mask stream
    mask_cols = [sum(len(parts) for _kc, _vs, parts in kcs) for kcs in keep]
    mask_off = [0]
    for qb in range(4):
        mask_off.append(mask_off[-1] + mask_cols[qb])
    n_mask_chunks = mask_off[-1]

    nc = bacc.Bacc("TRN2", target_bir_lowering=False, debug=False)
    xp = nc.dram_tensor("xp", [P, 4 * 16 * 512], bf16, kind="ExternalInput").ap()
    wq = nc.dram_tensor("wq", [P, 4 * 16 * P], bf16, kind="ExternalInput").ap()
    wk = nc.dram_tensor("wk", [P, 4 * 16 * P], bf16, kind="ExternalInput").ap()
    wv = nc.dram_tensor("wv", [P, 16 * DG], bf16, kind="ExternalInput").ap()
    wo = nc.dram_tensor("wo", [P, NG * H], bf16, kind="ExternalInput").ap()
    maskp = nc.dram_tensor("maskp", [P, max(n_mask_chunks, 1) * P], bf16,
                           kind="ExternalInput").ap()
    bqk = nc.dram_tensor("bqk", [P, 8], f32, kind="ExternalInput").ap()
    bvb = nc.dram_tensor("bvb", [P, DG], f32, kind="ExternalInput").ap()
    outp = nc.dram_tensor("outp", [S, H], bf16, kind="ExternalOutput").ap()

    NHC = H // P      # 16 contraction chunks over hidden
    NSQ = 4           # seq quarters for the projection phase
    SQ = S // NSQ     # 512
    NKC = S // P      # 16 key chunks
    NQB = 4           # query blocks
    QB = QBS          # 512
    ND = DG // P      # 4 d-chunks per section == heads per group

    # big/small interleave: full-length blocks alternate with short ones
    qb_iters = []
    for pair in ((3, 0), (2, 1)):
        for h in range(NG):
            qb_iters.append((pair[0], h))
            qb_iters.append((pair[1], h))

    with tile.TileContext(nc) as tc, ExitStack() as ctx:
        persist = ctx.enter_context(tc.tile_pool(name="persist", bufs=1))
        qT = persist.tile([P, NG * S], bf16)      # [d, h*S + s]
        kT = persist.tile([P, NG * S], bf16)      # [d, h*S + s]
        V = persist.tile([P, NKC * DG], bf16)     # [s, st*DG + d]
        woTs = persist.tile([P, NG * H], bf16)    # [d, h*H + n]
        bqk_s = persist.tile([P, 8], f32)
        bvb_s = persist.tile([P, DG], f32)
        ones = persist.tile([P, P], bf16)

        nc.vector.memset(ones[:], 1.0)

        mpool = ctx.enter_context(tc.tile_pool(name="mask", bufs=2))
        mask_tiles = {}

        def load_mask(qb):
            ncols = max(mask_cols[qb], 1)
            mtile = mpool.tile([P, ncols * P], bf16, tag="mt", name=f"mt{qb}")
            if mask_cols[qb]:
                nc.sync.dma_start(
                    out=mtile[:, :mask_cols[qb] * P],
                    in_=maskp[:, mask_off[qb] * P:mask_off[qb + 1] * P])
            mask_tiles[qb] = mtile

        # ---- Phase A: QKV projection ----
        # Weight slices stay resident in SBUF; x^T streams in seq quarters.
        with tc.tile_pool(name="wA", bufs=1) as wapool, \
             tc.tile_pool(name="xq", bufs=4) as xpool, \
             tc.tile_pool(name="pv_acc", bufs=1, space="PSUM") as pvp, \
             tc.tile_pool(name="pqk_acc", bufs=4, space="PSUM") as pqk:
            xq_tiles = {}

            def load_xq(sq, hf, split=1):
                t = xpool.tile([P, (NHC // 2) * SQ], bf16, tag="xq",
                               name=f"xq{sq}_{hf}")
                base = (sq * 2 + hf) * 4096
                step = 4096 // split
                for i in range(split):
                    nc.sync.dma_start(
                        out=t[:, i * step:(i + 1) * step],
                        in_=xp[:, base + i * step:base + (i + 1) * step])
                xq_tiles[(sq, hf)] = t

            # first weight chunk + first x quarter split fine so the first
            # matmul's wait covers ~100KB, not megabytes; non-critical loads
            # (biases, masks) are issued later so their ring entries don't
            # steal startup bandwidth
            # layout [h, hf*4096 + dc*1024 + (hc%8)*128 + d]: each hidden
            # half's weights contiguous, so DMAs land in need-order
            wq_sb = wapool.tile([P, ND * NHC * P], bf16)
            wv_sb = wapool.tile([P, NHC * DG], bf16)   # [h, hc*DG + d]
            wk_sb = wapool.tile([P, ND * NHC * P], bf16)
            nc.sync.dma_start(out=wq_sb[:, :512], in_=wq[:, :512])
            nc.sync.dma_start(out=wq_sb[:, 512:1024], in_=wq[:, 512:1024])
            load_xq(0, 0, split=4)
            nc.sync.dma_start(out=wq_sb[:, 1024:2048], in_=wq[:, 1024:2048])
            nc.sync.dma_start(out=wq_sb[:, 2048:4096], in_=wq[:, 2048:4096])
            nc.sync.dma_start(out=wv_sb[:, :4096], in_=wv[:, :4096])
            load_xq(0, 1, split=2)
            nc.sync.dma_start(out=wq_sb[:, 4096:], in_=wq[:, 4096:])
            nc.sync.dma_start(out=bqk_s[:], in_=bqk)
            nc.sync.dma_start(out=wv_sb[:, 4096:], in_=wv[:, 4096:])
            nc.sync.dma_start(out=bvb_s[:], in_=bvb)
            nc.sync.dma_start(out=wk_sb[:], in_=wk)

            for sq in range(NSQ):
                if sq == 2:
                    load_mask(3)
                    load_mask(0)
                for hf in range(2):
                    if (sq, hf) not in xq_tiles:
                        load_xq(sq, hf)
                xh = [xq_tiles.pop((sq, 0)), xq_tiles.pop((sq, 1))]
                if sq + 1 < NSQ:
                    load_xq(sq + 1, 0)
                    load_xq(sq + 1, 1)

                def xslice(hc, lo, hi):
                    return xh[hc // 8][:, (hc % 8) * SQ + lo:(hc % 8) * SQ + hi]

                # q and v accumulate in two hidden-dim halves so the first
                # matmuls only depend on the first x half-tile (faster ramp)
                qaccs = [pqk.tile([P, SQ], f32, tag="qkacc",
                                  name=f"qkacc{sq}_0_{dc}")
                         for dc in range(ND)]
                vaccs = [pvp.tile([P, DG], f32, tag=f"vacc{st}",
                                  name=f"vacc{st}_{sq}")
                         for st in range(4)]

                def q_half(hcs):
                    for dc in range(ND):
                        for hc in hcs:
                            col = (hc // 8) * 4096 + dc * 1024 + (hc % 8) * P
                            nc.tensor.matmul(
                                qaccs[dc][:],
                                lhsT=wq_sb[:, col: col + P],
                                rhs=xslice(hc, 0, SQ),
                                start=(hc == 0), stop=(hc == NHC - 1),
                            )
                        if hcs[-1] == NHC - 1:
                            nc.scalar.activation(
                                out=qT[:, dc * S + sq * SQ: dc * S + (sq + 1) * SQ],
                                in_=qaccs[dc][:], func=Identity,
                                bias=bqk_s[:, dc: dc + 1], scale=1.0,
                            )

                def v_half(hcs):
                    for hc in hcs:
                        for st in range(4):
                            nc.tensor.matmul(
                                vaccs[st][:],
                                lhsT=xslice(hc, st * P, (st + 1) * P),
                                rhs=wv_sb[:, hc * DG:(hc + 1) * DG],
                                start=(hc == 0), stop=(hc == NHC - 1),
                            )
                    if hcs[-1] == NHC - 1:
                        for st in range(4):
                            stg = sq * 4 + st
                            nc.vector.tensor_add(
                                V[:, stg * DG:(stg + 1) * DG], vaccs[st][:],
                                bvb_s[:])

                def k_sec():
                    for dc in range(ND):
                        acc = pqk.tile([P, SQ], f32, tag="qkacc",
                                       name=f"qkacc{sq}_1_{dc}")
                        for hc in range(NHC):
                            col = (hc // 8) * 4096 + dc * 1024 + (hc % 8) * P
                            nc.tensor.matmul(
                                acc[:],
                                lhsT=wk_sb[:, col: col + P],
                                rhs=xslice(hc, 0, SQ),
                                start=(hc == 0), stop=(hc == NHC - 1),
                            )
                        nc.scalar.activation(
                            out=kT[:, dc * S + sq * SQ: dc * S + (sq + 1) * SQ],
                            in_=acc[:], func=Identity,
                            bias=bqk_s[:, 4 + dc: 4 + dc + 1], scale=1.0,
                        )

                lo, hi = list(range(NHC // 2)), list(range(NHC // 2, NHC))
                if sq == NSQ - 1:
                    # k first on the last quarter so phase B's first scores
                    # matmuls (which need all of kT) unblock sooner
                    k_sec()
                    q_half(lo)
                    q_half(hi)
                    v_half(lo)
                    v_half(hi)
                else:
                    q_half(lo)
                    v_half(lo)
                    q_half(hi)
                    v_half(hi)
                    k_sec()

        # ---- Phase B+C: attention + output projection ----
        # Software-pipelined over (query-block, head): the QK->exp->mask
        # chain for iteration i+1 is emitted before the PV/r consumption of
        # iteration i.
        with tc.tile_pool(name="epool", bufs=4) as epool, \
             tc.tile_pool(name="cpool", bufs=2) as cpool, \
             tc.tile_pool(name="spool", bufs=2) as spool, \
             tc.tile_pool(name="tpool", bufs=2) as tpool, \
             tc.tile_pool(name="opool", bufs=2) as opool, \
             tc.tile_pool(name="ps_s", bufs=2, space="PSUM") as ps_s, \
             tc.tile_pool(name="ps_cr", bufs=1, space="PSUM") as ps_cr, \
             tc.tile_pool(name="ps_o", bufs=2, space="PSUM") as ps_o:
            e_tiles = {}
            e_roots = {}
            ctx_tiles = {}

            def rowsum_tree(qb, h, E, nchunks):
                """Pairwise-reduce E's chunks on DVE down to one [P,QB]
                bf16 tile; returns the root AP."""
                if nchunks == 1:
                    return E[:, :QB]
                scrA = tpool.tile([P, 8 * QB], bf16, tag="trA",
                                  name=f"trA{qb}_{h}")
                scrB = tpool.tile([P, 4 * QB], bf16, tag="trB",
                                  name=f"trB{qb}_{h}")
                src, srcn = E, nchunks
                dsts = [scrA, scrB]
                lvl = 0
                while srcn > 1:
                    half = srcn // 2
                    odd = srcn % 2
                    dst = dsts[lvl % 2]
                    nc.vector.tensor_add(
                        dst[:, :half * QB],
                        src[:, :half * QB],
                        src[:, half * QB:2 * half * QB])
                    if odd:
                        nc.vector.tensor_add(
                            dst[:, :QB], dst[:, :QB],
                            src[:, 2 * half * QB:(2 * half + 1) * QB])
                    src, srcn = dst, half
                    lvl += 1
                return src[:, :QB]

            def produce(qb, h):
                # mask prefetch: a slot is reused only after every read of
                # its previous tile has already been emitted
                if (qb, h) == (0, 3):
                    load_mask(2)
                if (qb, h) == (2, 0):
                    load_mask(1)
                mt = mask_tiles[qb]
                kcs = keep[qb]
                E = epool.tile([P, len(kcs) * QB], bf16, tag="E",
                               name=f"E{qb}_{h}")
                # zero the sub-128-granular invalid head of partial chunks
                # so the rowsum tree and the first PV stream read zeros
                for pos, (kc, vs, parts) in enumerate(kcs):
                    if vs:
                        nc.vector.memset(E[:, pos * QB:pos * QB + vs * P], 0.0)
                mpos = 0
                pos = 0
                n = len(kcs)
                while pos < n:
                    # pair-batch consecutive full chunks; partial chunks go
                    # one at a time at their valid width
                    if kcs[pos][1] == 0 and pos + 1 < n and kcs[pos + 1][1] == 0:
                        grp = [pos, pos + 1]
                    else:
                        grp = [pos]
                    ps = ps_s.tile([P, len(grp) * QB], f32, tag="ps",
                                   name=f"ps{qb}_{h}_{pos}")
                    for gi, p_ in enumerate(grp):
                        kc, vs, _parts = kcs[p_]
                        nc.tensor.matmul(
                            ps[:, gi * QB + vs * P:(gi + 1) * QB],
                            lhsT=kT[:, h * S + kc * P: h * S + (kc + 1) * P],
                            rhs=qT[:, h * S + qb * QB + vs * P:
                                   h * S + (qb + 1) * QB],
                            start=True, stop=True,
                        )
                    vs0 = kcs[grp[0]][1]
                    if len(grp) == 2:
                        nc.scalar.activation(
                            out=E[:, pos * QB:(pos + 2) * QB], in_=ps[:],
                            func=Exp, scale=SCALE)
                    else:
                        nc.scalar.activation(
                            out=E[:, pos * QB + vs0 * P:(pos + 1) * QB],
                            in_=ps[:, vs0 * P:QB], func=Exp, scale=SCALE)
                    for p_ in grp:
                        kc, vs, parts = kcs[p_]
                        for j in parts:
                            nc.vector.tensor_mul(
                                E[:, p_ * QB + j * P:p_ * QB + (j + 1) * P],
                                E[:, p_ * QB + j * P:p_ * QB + (j + 1) * P],
                                mt[:, mpos * P:(mpos + 1) * P])
                            mpos += 1
                    pos += len(grp)
                e_tiles[(qb, h)] = E
                e_roots[(qb, h)] = rowsum_tree(qb, h, E, n)

            def consume(qb, h):
                kcs = keep[qb]
                E = e_tiles.pop((qb, h))
                if h == 0:
                    ctx_tiles[qb] = cpool.tile(
                        [P, NG * QB], bf16, tag="ctxn", name=f"ctxn{qb}")
                ctxn = ctx_tiles[qb]
                pc = ps_cr.tile([P, QB], f32, tag="ctx", name=f"pc{qb}_{h}")
                pr = ps_cr.tile([P, QB], f32, tag="r", name=f"pr{qb}_{h}")
                last = len(kcs) - 1
                for pos, (kc, vs, _parts) in enumerate(kcs):
                    # first chunk streams full width (its invalid head is
                    # zeroed) so the whole PSUM tile gets initialized
                    w = 0 if pos == 0 else vs * P
                    nc.tensor.matmul(
                        pc[:, w:],
                        lhsT=V[:, kc * DG + h * P: kc * DG + (h + 1) * P],
                        rhs=E[:, pos * QB + w:(pos + 1) * QB],
                        start=(pos == 0), stop=(pos == last),
                    )
                root = e_roots.pop((qb, h))
                nc.tensor.matmul(pr[:], lhsT=ones[:], rhs=root,
                                 start=True, stop=True)
                rinv = spool.tile([P, QB], f32, tag="rinv", name=f"rinv{qb}_{h}")
                nc.vector.reciprocal_approx_fast(out=rinv[:], in_=pr[:])
                nc.vector.tensor_mul(
                    ctxn[:, h * QB:(h + 1) * QB], pc[:], rinv[:])

            def out_proj(qb):
                ctxn = ctx_tiles.pop(qb)
                for st in range(4):
                    ot = opool.tile([P, H], bf16, tag="ot", name=f"ot{qb}_{st}")
                    row = (qb * 4 + st) * P
                    for n in range(4):
                        po = ps_o.tile([P, 512], f32, tag="po",
                                       name=f"po{qb}_{st}_{n}")
                        for h in range(NG):
                            nc.tensor.matmul(
                                po[:],
                                lhsT=ctxn[:, h * QB + st * P: h * QB + (st + 1) * P],
                                rhs=woTs[:, h * H + n * 512: h * H + (n + 1) * 512],
                                start=(h == 0), stop=(h == NG - 1),
                            )
                        if n % 2 == 0:
                            nc.vector.tensor_copy(
                                ot[:, n * 512:(n + 1) * 512], po[:])
                        else:
                            nc.scalar.copy(ot[:, n * 512:(n + 1) * 512], po[:])
                        # half-tile DMAs so the drain overlaps the copies
                        if n == 1:
                            nc.sync.dma_start(out=outp[row:row + P, :1024],
                                              in_=ot[:, :1024])
                        elif n == 3:
                            nc.sync.dma_start(out=outp[row:row + P, 1024:],
                                              in_=ot[:, 1024:])

            nc.sync.dma_start(out=woTs[:], in_=wo)
            AHEAD = 3
            for j in range(AHEAD):
                produce(*qb_iters[j])
            for i, (qb, h) in enumerate(qb_iters):
                if i + AHEAD < len(qb_iters):
                    produce(*qb_iters[i + AHEAD])
                consume(qb, h)
                if h == NG - 1:
                    out_proj(qb)
    nc.compile()
    return nc


def _keep_lists(mask):
    """Per query-block: list of (kc, valid_start, parts) for key chunks
    whose [128k x 512q] mask block is not identically zero, classified at
    [128k x 128q] sub-block granularity:
      valid_start: first 128-q sub-block with any nonzero mask (the chunk's
        scores/E left of it are never computed; they are memset to zero).
      parts: sub-blocks >= valid_start that are not all-ones (these get the
        elementwise mask multiply).
    Exact for any float mask."""
    mt = mask.T.reshape(S // P, P, 4, 4, P)  # [kc, k, qb, qsub, q]
    bmax = mt.max(axis=(1, 4))  # [kc, qb, qsub]
    bmin = mt.min(axis=(1, 4))
    keep = []
    for qb in range(4):
        kcs = []
        for kc in range(S // P):
            zero = [bmax[kc, qb, j] == 0.0 for j in range(4)]
            onesb = [bmin[kc, qb, j] == 1.0 and bmax[kc, qb, j] == 1.0
                     for j in range(4)]
            if all(zero):
                continue
            vs = next(j for j in range(4) if not zero[j])
            parts = tuple(j for j in range(vs, 4) if not onesb[j])
            kcs.append((kc, vs, parts))
        keep.append(kcs if kcs else [(qb * 4, 0, (0, 1, 2, 3))])
    return keep


def _get_compiled(mask):
    keep = _keep_lists(mask)
    key = tuple(tuple(k) for k in keep)
    if key not in _COMPILED:
        _COMPILED[key] = (_build(keep), keep)
    return _COMPILED[key]


def _pack_pt(arr, inner):
    """[nchunk*128, n*inner] -> [128, n*nchunk*inner] with layout
    [p, n_idx*nchunk*inner + chunk*inner + i]."""
    nchunk = arr.shape[0] // P
    n = arr.shape[1] // inner
    return np.ascontiguousarray(
        arr.reshape(nchunk, P, n, inner).transpose(1, 2, 0, 3).reshape(
            P, n * nchunk * inner))


def _pack_qk(arr):
    """[2048h, 512d] -> [128, 8192] with layout
    [p, hf*4096 + dc*1024 + hcl*128 + d] (hc = hf*8 + hcl)."""
    return np.ascontiguousarray(
        arr.reshape(2, 8, P, 4, P).transpose(2, 0, 3, 1, 4).reshape(P, 8192))


def _in_maps(hidden_states, ltor_mask, W_qkv, b_qkv, W_out):
    bf = ml_dtypes.bfloat16
    hs = np.asarray(hidden_states, np.float32)
    mask = np.asarray(ltor_mask, np.float32).reshape(S, S)
    W_qkv = np.asarray(W_qkv, np.float32)
    b_qkv = np.asarray(b_qkv, np.float32)
    W_out = np.asarray(W_out, np.float32)

    keep = _keep_lists(mask)
    # mask^T packed: only the partial [128x128] sub-blocks, in keep order
    mT = mask.T.astype(bf)  # [k, q]
    cols = []
    for qb in range(4):
        for kc, _vs, parts in keep[qb]:
            for j in parts:
                cols.append(mT[kc * P:(kc + 1) * P,
                               qb * QBS + j * P:qb * QBS + (j + 1) * P])
    if cols:
        maskp = np.ascontiguousarray(
            np.concatenate(cols, axis=1).reshape(P, -1))
    else:
        maskp = np.zeros((P, P), bf)

    Wq, Wk, Wv = W_qkv[:H], W_qkv[H:2 * H], W_qkv[2 * H:]
    bq, bk, bv = b_qkv[:H], b_qkv[H:2 * H], b_qkv[2 * H:]

    # x^T packed per seq quarter: [p, sq*8192 + hc*512 + s]
    xps = [_pack_pt(hs[b].T.astype(bf), 512) for b in range(B)]
    in_maps = []
    for c in range(8):
        b, hg = divmod(c, NG)
        sl = slice(hg * DG, (hg + 1) * DG)
        bqk_np = np.concatenate(
            [bq[sl].reshape(4, P).T, bk[sl].reshape(4, P).T], axis=1)
        in_maps.append({
            "xp": xps[b],
            "wq": _pack_qk(Wq[sl].T.astype(bf)),  # [p, hf*4096+dc*1024+hcl*128+d]
            "wk": _pack_qk(Wk[sl].T.astype(bf)),
            "wv": _pack_pt(Wv[sl].T.astype(bf), DG),  # [p, hc*512+d]
            "wo": _pack_pt(W_out[:, sl].T.astype(bf), H),  # [p, h*2048+n]
            "maskp": maskp,
            "bqk": np.ascontiguousarray(bqk_np, dtype=np.float32),
            "bvb": np.ascontiguousarray(
                np.broadcast_to(bv[sl][None, :], (P, DG)), dtype=np.float32),
        })
    return in_maps


def kernel(hidden_states, ltor_mask, W_qkv, b_qkv, W_out, b_out):
    import os
    os.environ["BASS_NEVER_TRACE"] = "1"  # NTFF hook absent in this image
    from concourse.bass_utils import run_bass_kernel_spmd

    mask = np.asarray(ltor_mask, np.float32).reshape(S, S)
    nc, _ = _get_compiled(mask)
    in_maps = _in_maps(hidden_states, ltor_mask, W_qkv, b_qkv, W_out)
    res = run_bass_kernel_spmd(nc, in_maps, core_ids=list(range(8)))
    b_out = np.asarray(b_out, np.float32)
    out = np.empty((B, S, H), np.float32)
    for b in range(B):
        acc = res.results[NG * b]["outp"].astype(np.float32)
        for hg in range(1, NG):
            acc += res.results[NG * b + hg]["outp"].astype(np.float32)
        out[b] = acc + b_out[None, :]
    return out
